# revision 1
# baseline (speedup 1.0000x reference)
"""BlockTransformerPairBias Trainium2 kernel.

Sharding: 8 cores = (batch 0/1) x (4 groups of 16 attention blocks).
Each core computes its 1024 tokens end-to-end; no collectives.
Host slices inputs, folds LN/scale constants into weights, pre-transposes
the framepair slab to [cz, pairs] bf16, and concatenates core outputs.
"""

import sys

sys.path.insert(0, "/opt/trn_rl_repo")

from contextlib import ExitStack

import numpy as np
import ml_dtypes

import concourse.bass as bass
import concourse.tile as tile
from concourse import bacc, mybir
from concourse.bass_utils import run_bass_kernel_spmd
from concourse.masks import make_identity

F32 = mybir.dt.float32
BF16 = mybir.dt.bfloat16
I16 = mybir.dt.int16
AF = mybir.ActivationFunctionType
ALU = mybir.AluOpType
BF = ml_dtypes.bfloat16

B, N, NRES = 2, 4096, 1024
CS, CC, CZ, H, BLK = 512, 384, 128, 8, 64
CH = CS // H          # 64
NB = N // BLK         # 64
NCORES = 8
NBLK = NB * B // NCORES   # 16 blocks per core
NT = NBLK * BLK           # 1024 tokens per core
RT = NT // 128            # 8 token tiles
EPS = 1e-5

_CACHE = {}


def _declare(nc):
    t = {}

    def inp(name, shape, dt):
        t[name] = nc.dram_tensor(name, list(shape), dt, kind="ExternalInput").ap()

    inp("re", (NT, CS), F32)
    inp("zT", (NBLK, CZ, BLK * BLK), BF16)
    inp("s", (NRES, CC), F32)
    inp("idx", (128, NT // 16), I16)
    inp("wq", (128, 4, CS), BF16)
    inp("wk", (128, 4, CS), BF16)
    inp("wv", (128, 4, CS), BF16)
    inp("wg", (128, 4, CS), BF16)
    inp("wout", (128, 4, CS), BF16)
    inp("w1", (128, 4, 2 * CS), BF16)
    inp("w2", (128, 4, 2 * CS), BF16)
    inp("wb", (128, 8, CS), BF16)
    inp("wada", (128, 3, 3 * CS), BF16)
    inp("wbs", (CZ, 64), BF16)
    inp("svec", (H,), F32)          # holds MINUS S[h]
    inp("bq", (128, 4), F32)
    inp("bk", (128, 4), F32)
    inp("bada", (3 * CS,), F32)
    t["out"] = nc.dram_tensor("out", [NT, CS], F32, kind="ExternalOutput").ap()
    return t


def _bcast(ap, p=128):
    """Broadcast a 1-D DRAM AP across p partitions."""
    return bass.AP(tensor=ap.tensor, offset=ap.offset, ap=[[0, p]] + list(ap.ap))


def _ln_rstd(nc, sb, eps_t, x_ap):
    """bn stats over free dim -> (mean [P,1], rstd [P,1]) tiles."""
    p = x_ap.shape[0]
    stats = sb.tile([128, 6], F32, tag="stats")
    nc.vector.bn_stats(stats[:p], x_ap)
    mv = sb.tile([128, 2], F32, tag="mv")
    nc.vector.bn_aggr(mv[:p], stats[:p])
    sd = sb.tile([128, 1], F32, tag="sd")
    nc.scalar.activation(sd[:p], mv[:p, 1:2], AF.Sqrt, bias=eps_t[:p], scale=1.0)
    nc.vector.reciprocal(sd[:p], sd[:p])
    return mv[:p, 0:1], sd[:p]


def _emit(ctx, tc, t, flags):
    nc = tc.nc
    has_bq, has_bk, has_bag, has_bab, has_btg = flags

    consts = ctx.enter_context(tc.tile_pool(name="consts", bufs=1))
    hpool = ctx.enter_context(tc.tile_pool(name="hpool", bufs=1))
    sb = ctx.enter_context(tc.tile_pool(name="sb", bufs=2))
    ps_pt = ctx.enter_context(tc.tile_pool(name="ps_pt", bufs=1, space="PSUM"))
    ps_tp = ctx.enter_context(tc.tile_pool(name="ps_tp", bufs=2, space="PSUM"))

    # ---- constants / weights resident all kernel ----
    ident = consts.tile([128, 128], BF16)
    make_identity(nc, ident[:])
    eps_t = consts.tile([128, 1], F32)
    nc.vector.memset(eps_t[:], EPS)
    wbs_sb = consts.tile([CZ, 64], BF16)
    nc.sync.dma_start(wbs_sb[:], t["wbs"][:])
    svec_sb = consts.tile([128, H], F32)
    nc.sync.dma_start(svec_sb[:], _bcast(t["svec"]))
    idx_sb = consts.tile([128, NT // 16], I16)
    nc.sync.dma_start(idx_sb[:], t["idx"][:])
    bq_sb = consts.tile([128, 4], F32)
    bk_sb = consts.tile([128, 4], F32)
    if has_bq:
        nc.sync.dma_start(bq_sb[:], t["bq"][:])
    if has_bk:
        nc.sync.dma_start(bk_sb[:], t["bk"][:])
    wq = consts.tile([128, 4, CS], BF16)
    nc.sync.dma_start(wq[:], t["wq"][:])
    wk = consts.tile([128, 4, CS], BF16)
    nc.sync.dma_start(wk[:], t["wk"][:])
    wv = consts.tile([128, 4, CS], BF16)
    nc.sync.dma_start(wv[:], t["wv"][:])
    wg = consts.tile([128, 4, CS], BF16)
    nc.sync.dma_start(wg[:], t["wg"][:])
    wout = consts.tile([128, 4, CS], BF16)
    nc.sync.dma_start(wout[:], t["wout"][:])

    h_sb = hpool.tile([128, RT, CS], F32)

    dramp = ctx.enter_context(tc.tile_pool(name="dram", bufs=1, space="DRAM"))
    tbl = dramp.tile([NRES, 3 * CS], BF16)

    # ================= P1: cond tables =================
    with tc.tile_pool(name="p1", bufs=2) as p1p:
        wada = None
        with tc.tile_pool(name="p1w", bufs=1) as p1w:
            wada = p1w.tile([128, 3, 3 * CS], BF16)
            nc.sync.dma_start(wada[:], t["wada"][:])
            bada_bc = p1w.tile([128, 3 * CS], F32)
            if has_bag or has_bab or has_btg:
                nc.sync.dma_start(bada_bc[:], _bcast(t["bada"]))
            for r in range(NRES // 128):
                s_t = p1p.tile([128, CC], F32, tag="s_t")
                nc.sync.dma_start(s_t[:], t["s"][r * 128:(r + 1) * 128, :])
                mean, rstd = _ln_rstd(nc, sb, eps_t, s_t[:])
                cond = p1p.tile([128, CC], BF16, tag="cond")
                nc.vector.tensor_scalar(out=cond[:], in0=s_t[:], scalar1=mean,
                                        scalar2=rstd, op0=ALU.subtract, op1=ALU.mult)
                ct = p1p.tile([128, 3, 128], BF16, tag="ct")
                for c in range(3):
                    tp = ps_tp.tile([128, 128], BF16, tag="tp")
                    nc.tensor.transpose(tp[:], cond[:, c * 128:(c + 1) * 128], ident[:])
                    nc.scalar.copy(ct[:, c, :], tp[:])
                tbl_sb = p1p.tile([128, 3 * CS], BF16, tag="tbl_sb")
                for n in range(3):
                    pt = ps_pt.tile([128, CS], F32, tag="pt")
                    for k in range(3):
                        nc.tensor.matmul(pt[:], ct[:, k, :],
                                         wada[:, k, n * CS:(n + 1) * CS],
                                         start=(k == 0), stop=(k == 2))
                    seg = slice(n * CS, (n + 1) * CS)
                    if n == 0:
                        if has_bag:
                            nc.vector.tensor_add(pt[:], pt[:], bada_bc[:, seg])
                        nc.scalar.activation(tbl_sb[:, seg], pt[:], AF.Sigmoid)
                    elif n == 1:
                        if has_bab:
                            nc.vector.tensor_add(tbl_sb[:, seg], pt[:], bada_bc[:, seg])
                        else:
                            nc.vector.tensor_copy(tbl_sb[:, seg], pt[:])
                    else:
                        if has_btg:
                            nc.vector.tensor_add(pt[:], pt[:], bada_bc[:, seg])
                        nc.scalar.activation(tbl_sb[:, seg], pt[:], AF.Sigmoid)
                nc.sync.dma_start(tbl[r * 128:(r + 1) * 128, :], tbl_sb[:])

    # ============ P3..P7: LN1, projections, bias path, attention ============
    with tc.tile_pool(name="attacts", bufs=1) as aa:
        re = aa.tile([128, RT, CS], F32)
        nc.sync.dma_start(re[:], t["re"].rearrange("(r p) c -> p r c", p=128))
        xnT = aa.tile([128, 4, NT], BF16)
        for r in range(RT):
            mean, rstd = _ln_rstd(nc, sb, eps_t, re[:, r, :])
            xn = sb.tile([128, CS], BF16, tag="xn")
            nc.vector.tensor_scalar(out=xn[:], in0=re[:, r, :], scalar1=mean,
                                    scalar2=rstd, op0=ALU.subtract, op1=ALU.mult)
            for c in range(4):
                tp = ps_tp.tile([128, 128], BF16, tag="tp")
                nc.tensor.transpose(tp[:], xn[:, c * 128:(c + 1) * 128], ident[:])
                nc.scalar.copy(xnT[:, c, r * 128:(r + 1) * 128], tp[:])

        # ---- projections ----
        qf = aa.tile([128, 4, NT], BF16)
        kf = aa.tile([128, 4, NT], BF16)
        for (w, bias_sb, has_b, dst) in ((wq, bq_sb, has_bq, qf),
                                         (wk, bk_sb, has_bk, kf)):
            for m in range(4):
                for n in range(2):
                    pt = ps_pt.tile([128, CS], F32, tag="pt")
                    for k in range(4):
                        nc.tensor.matmul(pt[:], w[:, k, m * 128:(m + 1) * 128],
                                         xnT[:, k, n * 512:(n + 1) * 512],
                                         start=(k == 0), stop=(k == 3))
                    dseg = dst[:, m, n * 512:(n + 1) * 512]
                    if has_b:
                        nc.vector.tensor_scalar_add(out=dseg, in0=pt[:],
                                                    scalar1=bias_sb[:, m:m + 1])
                    else:
                        nc.vector.tensor_copy(dseg, pt[:])
        # odd heads' q/k rows duplicated at partition base 0: every QK matmul
        # then issues from PE row-group 0 (mixed row-groups draining into one
        # PSUM bank concurrently crash the device)
        qf2 = aa.tile([64, 4, NT], BF16)
        nc.sync.dma_start(qf2[:], qf[64:128, :, :])
        kf2 = aa.tile([64, 4, NT], BF16)
        nc.sync.dma_start(kf2[:], kf[64:128, :, :])
        vtm = aa.tile([64, NBLK, CS], BF16)
        gsig = aa.tile([128, RT, CS], BF16)
        for g in range(NBLK):
            pt = ps_pt.tile([128, CS], F32, tag="pt")
            for k in range(4):
                nc.tensor.matmul(pt[0:64, :], xnT[:, k, g * 64:(g + 1) * 64],
                                 wv[:, k, :], start=(k == 0), stop=(k == 3))
            nc.vector.tensor_copy(vtm[:, g, :], pt[0:64, :])
        for r in range(RT):
            pt2 = ps_pt.tile([128, CS], F32, tag="pt")
            for k in range(4):
                nc.tensor.matmul(pt2[:], xnT[:, k, r * 128:(r + 1) * 128],
                                 wg[:, k, :], start=(k == 0), stop=(k == 3))
            nc.scalar.activation(gsig[:, r, :], pt2[:], AF.Sigmoid)

        # ---- bias path + attention + Wout, per block pair ----
        from concourse.tile import add_dep_helper
        with tc.tile_pool(name="big", bufs=2) as big, \
             tc.tile_pool(name="z2p", bufs=1) as z2p, \
             tc.tile_pool(name="dramP", bufs=2, space="DRAM") as dpp, \
             tc.tile_pool(name="ps_pz", bufs=2, space="PSUM") as ps_pz, \
             tc.tile_pool(name="ps_sc", bufs=1, space="PSUM") as ps_sc, \
             tc.tile_pool(name="ps_at", bufs=1, space="PSUM") as ps_at, \
             tc.tile_pool(name="ps_o", bufs=1, space="PSUM") as ps_o:
            prev_lds = {}
            for gp in range(RT):
                Pr = sb.tile([128, 10, 64], F32, tag="Pr")
                for g2 in range(2):
                    g = 2 * gp + g2
                    zt = big.tile([CZ, BLK * BLK], BF16, tag="zt")
                    nc.gpsimd.dma_start(zt[:], t["zT"][g])
                    z2 = z2p.tile([CZ, BLK * BLK], BF16, tag="z2")
                    nc.vector.tensor_mul(z2[:], zt[:], zt[:])
                    Psbb = big.tile([128, 1024], F32, tag="Psbb")
                    ze = ps_pz.tile([128, 512], F32, tag="pz")
                    zo = ps_pz.tile([128, 512], F32, tag="pz")
                    for cg in range(4):
                        tpos = (0, 32 * cg)
                        rows = slice(32 * cg, 32 * cg + 32)
                        ev = slice((2 * cg) * 512, (2 * cg + 1) * 512)
                        od = slice((2 * cg + 1) * 512, (2 * cg + 2) * 512)
                        # z pass writes P rows 0..8; z^2 pass accumulates into
                        # row 9 via a shifted ones column (start=False).
                        nc.tensor.matmul(ze[rows, :], wbs_sb[:, 0:32], zt[:, ev],
                                         start=True, stop=False, tile_position=tpos)
                        nc.tensor.matmul(zo[rows, :], wbs_sb[:, 0:32], zt[:, od],
                                         start=True, stop=False, tile_position=tpos)
                        nc.tensor.matmul(ze[rows, :], wbs_sb[:, 32:64], z2[:, ev],
                                         start=False, stop=True, tile_position=tpos)
                        nc.tensor.matmul(zo[rows, :], wbs_sb[:, 32:64], z2[:, od],
                                         start=False, stop=True, tile_position=tpos)
                    nc.scalar.copy(Psbb[:, 0:512], ze[:])
                    nc.vector.tensor_copy(Psbb[:, 512:1024], zo[:])
                    # round-trip through DRAM to reshape [32cg+m, (ab i3 j)]
                    # -> [i=(cg ab i3), m, j]
                    dP = dpp.tile([128, 1024], F32, tag="dP")
                    st = nc.gpsimd.dma_start(dP[:], Psbb[:])
                    for l in prev_lds.get(g % 2, ()):
                        add_dep_helper(st.ins, l, reason="dramP WAR")
                    base = dP[:]
                    lds = []
                    for cg in range(4):
                        src = bass.AP(tensor=base.tensor,
                                      offset=base.offset + cg * 32768,
                                      ap=[[64, 16], [1024, 10], [1, 64]])
                        ld = nc.gpsimd.dma_start(
                            Pr[g2 * 64 + cg * 16:g2 * 64 + (cg + 1) * 16, :, :],
                            src)
                        add_dep_helper(ld.ins, st.ins, reason="reshape RAW")
                        lds.append(ld.ins)
                    prev_lds[g % 2] = lds

                # stats for the pair: mean in Pr[:,8], E[z^2] in Pr[:,9]
                msq = sb.tile([128, 64], F32, tag="msq")
                nc.vector.tensor_mul(msq[:], Pr[:, 8, :], Pr[:, 8, :])
                var_t = sb.tile([128, 64], F32, tag="var_t")
                nc.vector.tensor_sub(var_t[:], Pr[:, 9, :], msq[:])
                nc.scalar.activation(var_t[:], var_t[:], AF.Sqrt,
                                     bias=eps_t[:], scale=1.0)
                rstd_t = sb.tile([128, 64], F32, tag="rstd_t")
                nc.vector.reciprocal(rstd_t[:], var_t[:])
                mr_t = sb.tile([128, 64], F32, tag="mr_t")
                nc.vector.tensor_mul(mr_t[:], Pr[:, 8, :], rstd_t[:])

                def b0(ap_, reps, at=None):
                    lst = list(ap_.ap)
                    pos = len(lst) if at is None else at
                    lst.insert(pos, [0, reps])
                    return bass.AP(tensor=ap_.tensor, offset=ap_.offset, ap=lst)

                # bias_all[p,(h,j)] = Pr_h*rstd - S_h*mean*rstd  (svec = -S)
                mrs = sb.tile([128, H, 64], F32, tag="mrs")
                nc.vector.tensor_mul(mrs[:], b0(mr_t[:], H, at=1), b0(svec_sb[:], 64))
                bias_all = sb.tile([128, H, 64], F32, tag="bias_all")
                nc.vector.tensor_mul(bias_all[:], Pr[:, 0:H, :],
                                     b0(rstd_t[:], H, at=1))
                nc.vector.tensor_add(bias_all[:], bias_all[:], mrs[:])

                # ---- attention: all heads, both blocks ----
                sc_ps = ps_sc.tile([128, CS], F32, tag="sc_ps")
                for g2 in range(2):
                    g = 2 * gp + g2
                    for h in range(H):
                        m = h // 2
                        qsl = (qf[0:64, m, g * 64:(g + 1) * 64] if h % 2 == 0
                               else qf2[:, m, g * 64:(g + 1) * 64])
                        ksl = (kf[0:64, m, g * 64:(g + 1) * 64] if h % 2 == 0
                               else kf2[:, m, g * 64:(g + 1) * 64])
                        nc.tensor.matmul(sc_ps[g2 * 64:g2 * 64 + 64,
                                               h * 64:(h + 1) * 64],
                                         qsl, ksl, start=True, stop=True,
                                         tile_position=(0, g2 * 64))
                sc_sb = sb.tile([128, CS], F32, tag="sc_sb")
                nc.vector.tensor_add(sc_sb[:].rearrange("p (h j) -> p h j", h=H),
                                     sc_ps[:].rearrange("p (h j) -> p h j", h=H),
                                     bias_all[:])
                a_sb = sb.tile([128, CS], BF16, tag="a_sb")
                nc.scalar.activation(a_sb[:], sc_sb[:], AF.Exp)
                rs = sb.tile([128, H], F32, tag="rs")
                nc.vector.tensor_reduce(rs[:], a_sb[:].rearrange(
                    "p (h j) -> p h j", h=H), axis=mybir.AxisListType.X, op=ALU.add)
                rcp = sb.tile([128, H], F32, tag="rcp")
                nc.vector.reciprocal(rcp[:], rs[:])

                o_ps = ps_o.tile([128, CS], F32, tag="o_ps")
                for g2 in range(2):
                    g = 2 * gp + g2
                    idq = ident[g2 * 64:g2 * 64 + 64, g2 * 64:g2 * 64 + 64]
                    aT_ps = ps_at.tile([64, CS], BF16, tag="aT_ps")
                    for h in range(H):
                        nc.tensor.transpose(aT_ps[:, h * 64:(h + 1) * 64],
                                            a_sb[g2 * 64:g2 * 64 + 64,
                                                 h * 64:(h + 1) * 64], idq)
                    aT_sb = sb.tile([64, CS], BF16, tag="aT_sb")
                    nc.scalar.copy(aT_sb[:], aT_ps[:])
                    for h in range(H):
                        nc.tensor.matmul(
                            o_ps[g2 * 64:g2 * 64 + 64, h * 64:(h + 1) * 64],
                            aT_sb[:, h * 64:(h + 1) * 64],
                            vtm[:, g, h * 64:(h + 1) * 64],
                            start=True, stop=True, tile_position=(0, g2 * 64))
                o_n = sb.tile([128, CS], BF16, tag="o_n")
                nc.vector.tensor_mul(o_n[:].rearrange("p (h j) -> p h j", h=H),
                                     o_ps[:].rearrange("p (h j) -> p h j", h=H),
                                     b0(rcp[:], 64))
                og_pair = sb.tile([128, CS], BF16, tag="og_pair")
                nc.vector.tensor_mul(og_pair[:], o_n[:], gsig[:, gp, :])
                ogT = sb.tile([128, 4, 128], BF16, tag="ogT")
                for c in range(4):
                    tp = ps_tp.tile([128, 128], BF16, tag="tp")
                    nc.tensor.transpose(tp[:], og_pair[:, c * 128:(c + 1) * 128],
                                        ident[:])
                    nc.scalar.copy(ogT[:, c, :], tp[:])
                # ---- Wout + residual ----
                pt = ps_pt.tile([128, CS], F32, tag="pt")
                for k in range(4):
                    nc.tensor.matmul(pt[:], ogT[:, k, :], wout[:, k, :],
                                     start=(k == 0), stop=(k == 3))
                nc.vector.tensor_add(h_sb[:, gp, :], pt[:], re[:, gp, :])

    # ================= P2+P8..P10: gather, transition =================
    with tc.tile_pool(name="acts2", bufs=1) as a2:
        gth = a2.tile([128, RT, 3 * CS], BF16)
        for r in range(RT):
            nc.gpsimd.dma_gather(
                out_ap=gth[:, r:r + 1, :], in_ap=tbl[:],
                idxs_ap=idx_sb[:, r * 8:(r + 1) * 8],
                num_idxs=128, num_idxs_reg=128, elem_size=3 * CS)

        tT = a2.tile([128, 4, NT], BF16)
        for r in range(RT):
            mean, rstd = _ln_rstd(nc, sb, eps_t, h_sb[:, r, :])
            t0 = sb.tile([128, CS], BF16, tag="t0")
            nc.vector.tensor_scalar(out=t0[:], in0=h_sb[:, r, :], scalar1=mean,
                                    scalar2=rstd, op0=ALU.subtract, op1=ALU.mult)
            t1 = sb.tile([128, CS], BF16, tag="t1")
            nc.vector.tensor_mul(t1[:], t0[:], gth[:, r, 0:CS])
            t2 = sb.tile([128, CS], BF16, tag="t2")
            nc.vector.tensor_add(t2[:], t1[:], gth[:, r, CS:2 * CS])
            for c in range(4):
                tp = ps_tp.tile([128, 128], BF16, tag="tp")
                nc.tensor.transpose(tp[:], t2[:, c * 128:(c + 1) * 128], ident[:])
                nc.scalar.copy(tT[:, c, r * 128:(r + 1) * 128], tp[:])

        w1 = a2.tile([128, 4, 2 * CS], BF16)
        nc.sync.dma_start(w1[:], t["w1"][:])
        w2 = a2.tile([128, 4, 2 * CS], BF16)
        nc.sync.dma_start(w2[:], t["w2"][:])
        wb = a2.tile([128, 8, CS], BF16)
        nc.sync.dma_start(wb[:], t["wb"][:])
        bb = a2.tile([128, 8, NT], BF16)
        for m in range(8):
            for n in range(2):
                p1 = ps_pt.tile([128, CS], F32, tag="pt")
                for k in range(4):
                    nc.tensor.matmul(p1[:], w1[:, k, m * 128:(m + 1) * 128],
                                     tT[:, k, n * 512:(n + 1) * 512],
                                     start=(k == 0), stop=(k == 3))
                u1s = sb.tile([128, 512], F32, tag="u1s")
                nc.scalar.activation(u1s[:], p1[:], AF.Sigmoid)
                u1 = sb.tile([128, 512], F32, tag="u1")
                nc.vector.tensor_mul(u1[:], u1s[:], p1[:])
                p2 = ps_pt.tile([128, CS], F32, tag="pt")
                for k in range(4):
                    nc.tensor.matmul(p2[:], w2[:, k, m * 128:(m + 1) * 128],
                                     tT[:, k, n * 512:(n + 1) * 512],
                                     start=(k == 0), stop=(k == 3))
                nc.vector.tensor_mul(bb[:, m, n * 512:(n + 1) * 512], u1[:], p2[:])

        for r in range(RT):
            pt = ps_pt.tile([128, CS], F32, tag="pt")
            for k in range(8):
                nc.tensor.matmul(pt[:], bb[:, k, r * 128:(r + 1) * 128], wb[:, k, :],
                                 start=(k == 0), stop=(k == 7))
            tg32 = sb.tile([128, CS], F32, tag="tg32")
            nc.scalar.copy(tg32[:], gth[:, r, 2 * CS:3 * CS])
            tr = sb.tile([128, CS], F32, tag="tr")
            nc.vector.tensor_mul(tr[:], pt[:], tg32[:])
            out_t = sb.tile([128, CS], F32, tag="out_t")
            nc.vector.tensor_add(out_t[:], tr[:], h_sb[:, r, :])
            nc.sync.dma_start(t["out"][r * 128:(r + 1) * 128, :], out_t[:])


def build(flags):
    key = ("v1", flags)
    if key in _CACHE:
        return _CACHE[key]
    nc = bacc.Bacc("TRN2", target_bir_lowering=False, debug=False)
    t = _declare(nc)
    with tile.TileContext(nc) as tc:
        with ExitStack() as ctx:
            _emit(ctx, tc, t, flags)
    nc.compile()
    _CACHE[key] = nc
    return nc


def prep_core_inputs(inputs, core):
    """Host-side slicing + weight folding for one core."""
    b = core // 4
    g0 = (core % 4) * NBLK
    r0 = g0 * BLK

    f = lambda k: np.asarray(inputs[k], np.float32)
    ln_w, ln_b = f("ln_w"), f("ln_b")
    sc = 1.0 / np.sqrt(CH)

    def fold(w, scale=1.0):
        return ln_w[:, None] * np.asarray(w, np.float32) * scale

    def foldb(w, scale=1.0):
        return (ln_b @ np.asarray(w, np.float32)) * scale

    Wkv = f("Wkv")
    wq_h, bq_h = fold(inputs["Wq"], sc), foldb(inputs["Wq"], sc)
    wk_h, bk_h = fold(Wkv[:, :CS]), foldb(Wkv[:, :CS])
    wv_h, bv_h = fold(Wkv[:, CS:]), foldb(Wkv[:, CS:])
    wg_h, bg_h = fold(inputs["Wgate"]), foldb(inputs["Wgate"])
    if np.any(bv_h) or np.any(bg_h):
        raise NotImplementedError("nonzero folded v/gate bias unsupported")

    cw = f("adaln_cond_w")
    wada_h = np.concatenate(
        [cw[:, None] * f("W_ada_gate"), cw[:, None] * f("W_ada_bias"),
         cw[:, None] * f("W_tgate")], axis=1)
    bada_h = np.concatenate(
        [f("b_ada_gate"), np.zeros(CS, np.float32), f("b_tgate")]).astype(np.float32)

    wbias = f("bias_ln_w")[:, None] * f("Wbias")      # [128, 8]
    svec_h = (-wbias.sum(0)).astype(np.float32)       # minus S
    wbs_h = np.zeros((CZ, 64), np.float32)
    wbs_h[:, :H] = wbias
    wbs_h[:, 8] = 1.0 / CZ       # sum column directly produces the mean
    wbs_h[:, 32 + 9] = 1.0 / CZ  # z^2 pass accumulates E[z^2] into row 9

    def ktile(w, kt):
        w = np.asarray(w, np.float32)
        return np.ascontiguousarray(
            w.reshape(kt, 128, w.shape[1]).transpose(1, 0, 2)).astype(BF)

    # framepair: [16, 64, 64, 128] -> [16, 128, 4096] bf16
    fp = np.asarray(inputs["framepair_embed"][b, g0:g0 + NBLK], np.float32)
    zT = np.ascontiguousarray(
        fp.reshape(NBLK, BLK * BLK, CZ).transpose(0, 2, 1)).astype(BF)

    idx = np.asarray(inputs["rigids_to_res_idx"][b, r0:r0 + NT]).astype(np.int16)
    idx_w = np.empty((128, NT // 16), np.int16)
    for p in range(16):
        idx_w[p] = idx[p::16]
    idx_w[16:] = np.tile(idx_w[:16], (7, 1))

    return {
        "re": np.ascontiguousarray(inputs["rigids_embed"][b, r0:r0 + NT]).astype(np.float32),
        "zT": zT,
        "s": np.ascontiguousarray(inputs["s"][b]).astype(np.float32),
        "idx": idx_w,
        "wq": ktile(wq_h, 4), "wk": ktile(wk_h, 4), "wv": ktile(wv_h, 4),
        "wg": ktile(wg_h, 4), "wout": ktile(inputs["Wout"], 4),
        "w1": ktile(inputs["W1"], 4), "w2": ktile(inputs["W2"], 4),
        "wb": ktile(inputs["Wb"], 8), "wada": ktile(wada_h, 3),
        "wbs": wbs_h.astype(BF), "svec": svec_h,
        "bq": np.ascontiguousarray(bq_h.reshape(4, 128).T),
        "bk": np.ascontiguousarray(bk_h.reshape(4, 128).T),
        "bada": bada_h,
    }, (bool(np.any(bq_h)), bool(np.any(bk_h)), bool(np.any(f("b_ada_gate"))),
        False, bool(np.any(f("b_tgate"))))


def kernel(**inputs):
    mask = np.asarray(inputs["rigids_mask"])
    if not np.all(mask == 1.0):
        print("WARNING: rigids_mask not all ones; kernel assumes ones", file=sys.stderr)

    in_maps, flags = [], None
    for core in range(NCORES):
        m, flags = prep_core_inputs(inputs, core)
        in_maps.append(m)

    nc = build(flags)
    res = run_bass_kernel_spmd(nc, in_maps, core_ids=list(range(NCORES)))

    out = np.empty((B, N, CS), np.float32)
    for core in range(NCORES):
        b = core // 4
        r0 = (core % 4) * NT
        out[b, r0:r0 + NT] = res.results[core]["out"]
    return out



# revision 11
# speedup vs baseline: 1.7899x; 1.7899x over previous
"""BlockTransformerPairBias Trainium2 kernel (v2 — pipelined).

Sharding: 8 cores = (batch 0/1) x (4 groups of 16 attention blocks).
Each core computes its 1024 tokens end-to-end; no collectives.

v2 restructure vs baseline:
- PSUM/SBUF double buffering everywhere (no bufs=1 serialization)
- batched LN statistics; ACT pinned to one transcendental per phase
  (table reloads cost 46us in the baseline); PSUM->SBUF copies ride
  ACT's Copy path (no table) or DVE
- bias-path weights pre-folded (wbs' = w - S/128) so the pair bias is
  just P'*rstd; the P reshape round-trip is batched per 8-block chunk
- attention software-pipelined across block-pairs (QK of gp+1 emitted
  before the tail of gp); V projection 2-way column-tiled
- re/h kept in bf16; zT loads split across sync+gpsimd DMA queues
"""

import sys

sys.path.insert(0, "/opt/trn_rl_repo")

from contextlib import ExitStack

import numpy as np
import ml_dtypes

import concourse.bass as bass
import concourse.tile as tile
from concourse import bacc, mybir
from concourse.bass_utils import run_bass_kernel_spmd
from concourse.masks import make_identity
from concourse.tile import add_dep_helper

F32 = mybir.dt.float32
BF16 = mybir.dt.bfloat16
I16 = mybir.dt.int16
AF = mybir.ActivationFunctionType
ALU = mybir.AluOpType
BF = ml_dtypes.bfloat16

B, N, NRES = 2, 4096, 1024
CS, CC, CZ, H, BLK = 512, 384, 128, 8, 64
CH = CS // H          # 64
NB = N // BLK         # 64
NCORES = 8
NBLK = NB * B // NCORES   # 16 blocks per core
NT = NBLK * BLK           # 1024 tokens per core
RT = NT // 128            # 8 token tiles
EPS = 1e-5

_CACHE = {}


def _declare(nc):
    t = {}

    def inp(name, shape, dt):
        t[name] = nc.dram_tensor(name, list(shape), dt, kind="ExternalInput").ap()

    inp("re", (NT, CS), BF16)
    inp("zT", (NBLK, CZ, BLK * BLK), BF16)
    inp("s", (NRES, CC), F32)
    inp("idx", (128, NT // 16), I16)
    inp("wq", (128, 4, CS), BF16)
    inp("wk", (128, 4, CS), BF16)
    inp("wv", (128, 4, CS), BF16)
    inp("wg", (128, 4, CS), BF16)
    inp("wout", (128, 4, CS), BF16)
    inp("w1", (128, 4, 2 * CS), BF16)
    inp("w2", (128, 4, 2 * CS), BF16)
    inp("wb", (128, 8, CS), BF16)
    inp("wada", (128, 3, 3 * CS), BF16)
    inp("wbs", (CZ, 64), BF16)
    inp("bq", (128, 4), F32)
    inp("bk", (128, 4), F32)
    inp("bada", (3 * CS,), F32)
    t["out"] = nc.dram_tensor("out", [NT, CS], F32, kind="ExternalOutput").ap()
    return t


def _bcast(ap, p=128):
    """Broadcast a 1-D DRAM AP across p partitions."""
    return bass.AP(tensor=ap.tensor, offset=ap.offset, ap=[[0, p]] + list(ap.ap))


def _b0(ap_, reps, at=None):
    """Insert a 0-stride broadcast dim into a free position of an AP."""
    lst = list(ap_.ap)
    pos = len(lst) if at is None else at
    lst.insert(pos, [0, reps])
    return bass.AP(tensor=ap_.tensor, offset=ap_.offset, ap=lst)


def _batch_stats(nc, sb, src_rc, nrt, tagp):
    """LN stats for nrt row-tiles: returns (mv [128,nrt,2], rstd [128,nrt]).

    One ACT Sqrt for the whole batch (rstd = sqrt(1/(var+eps)) via DVE
    reciprocal_approx_fast), so the ACT table loads once per phase.
    src_rc(r) must return the [128, C] AP of tile r.
    """
    st6 = sb.tile([128, nrt, 6], F32, tag=tagp + "st6")
    for r in range(nrt):
        nc.vector.bn_stats(st6[:, r, :], src_rc(r))
    mv = sb.tile([128, nrt, 2], F32, tag=tagp + "mv")
    for r in range(nrt):
        nc.vector.bn_aggr(mv[:, r, :], st6[:, r, :])
    vc = sb.tile([128, nrt], F32, tag=tagp + "vc")
    nc.vector.tensor_scalar_add(
        vc[:], mv[:, :, 1:2].rearrange("p r o -> p (r o)"), EPS)
    rv = sb.tile([128, nrt], F32, tag=tagp + "rv")
    nc.vector.reciprocal_approx_fast(out=rv[:], in_=vc[:])
    rstd = sb.tile([128, nrt], F32, tag=tagp + "rstd")
    nc.scalar.activation(rstd[:], rv[:], AF.Sqrt)
    return mv, rstd


def _emit(ctx, tc, t, flags):
    nc = tc.nc
    has_bq, has_bk, has_bag, btg_const, has_btg = flags

    consts = ctx.enter_context(tc.tile_pool(name="consts", bufs=1))
    top = ctx.enter_context(tc.tile_pool(name="top", bufs=1))
    sb = ctx.enter_context(tc.tile_pool(name="sb", bufs=2))
    dramp = ctx.enter_context(tc.tile_pool(name="dram", bufs=1, space="DRAM"))

    # ---- constants / weights resident all kernel ----
    ident = consts.tile([128, 128], BF16)
    make_identity(nc, ident[:])
    wbs_sb = consts.tile([CZ, 64], BF16)
    nc.sync.dma_start(wbs_sb[:], t["wbs"][:])
    idx_sb = consts.tile([128, NT // 16], I16)
    nc.sync.dma_start(idx_sb[:], t["idx"][:])
    bq_sb = consts.tile([128, 4], F32)
    bk_sb = consts.tile([128, 4], F32)
    if has_bq:
        nc.sync.dma_start(bq_sb[:], t["bq"][:])
    if has_bk:
        nc.sync.dma_start(bk_sb[:], t["bk"][:])
    wq = consts.tile([128, 4, CS], BF16)
    nc.sync.dma_start(wq[:], t["wq"][:])
    wk = consts.tile([128, 4, CS], BF16)
    nc.sync.dma_start(wk[:], t["wk"][:])
    wv = consts.tile([128, 4, CS], BF16)
    nc.sync.dma_start(wv[:], t["wv"][:])
    wg = consts.tile([128, 4, CS], BF16)
    nc.sync.dma_start(wg[:], t["wg"][:])
    wout = consts.tile([128, 4, CS], BF16)
    nc.sync.dma_start(wout[:], t["wout"][:])
    btg_t = consts.tile([128, 1], F32)
    if btg_const:
        nc.vector.memset(btg_t[:], btg_const)

    re_sb = top.tile([128, RT, CS], BF16)
    nc.sync.dma_start(re_sb[:], t["re"].rearrange("(r p) c -> p r c", p=128))
    h_sb = top.tile([128, RT, CS], BF16)
    gth = top.tile([128, RT, 3 * CS], BF16)

    tbl = dramp.tile([NRES, 3 * CS], BF16)
    pr_d = dramp.tile([10, NBLK * BLK * BLK], BF16)

    # ================= P1: cond tables =================
    with tc.tile_pool(name="p1", bufs=2) as p1p, \
         tc.tile_pool(name="p1w", bufs=1) as p1w, \
         tc.tile_pool(name="ps_tp1", bufs=2, space="PSUM") as ps_tp1, \
         tc.tile_pool(name="ps_p1", bufs=3, space="PSUM") as ps_p1:
        wada = p1w.tile([128, 3, 3 * CS], BF16)
        nc.sync.dma_start(wada[:], t["wada"][:])
        bada_bc = p1w.tile([128, 3 * CS], F32)
        if has_bag or (has_btg and btg_const is None):
            nc.sync.dma_start(bada_bc[:], _bcast(t["bada"]))
        s_all = p1w.tile([128, NRES // 128, CC], F32)
        nc.gpsimd.dma_start(s_all[:], t["s"].rearrange("(r p) c -> p r c", p=128))

        mv1, rstd1 = _batch_stats(nc, sb, lambda r: s_all[:, r, :], 8, "p1")
        for r in range(NRES // 128):
            cond = p1p.tile([128, CC], BF16, tag="cond")
            nc.vector.tensor_scalar(out=cond[:], in0=s_all[:, r, :],
                                    scalar1=mv1[:, r, 0:1],
                                    scalar2=rstd1[:, r:r + 1],
                                    op0=ALU.subtract, op1=ALU.mult)
            tp = ps_tp1.tile([128, CC], BF16, tag="tp1")
            for c in range(3):
                nc.tensor.transpose(tp[:, c * 128:(c + 1) * 128],
                                    cond[:, c * 128:(c + 1) * 128], ident[:])
            ct = p1p.tile([128, 3, 128], BF16, tag="ct")
            nc.vector.tensor_copy(ct[:].rearrange("p k c -> p (k c)"), tp[:])
            tbl_sb = p1p.tile([128, 3 * CS], BF16, tag="tblsb")
            for n in range(3):
                pt = ps_p1.tile([128, CS], F32, tag="p1pt")
                for k in range(3):
                    nc.tensor.matmul(pt[:], ct[:, k, :],
                                     wada[:, k, n * CS:(n + 1) * CS],
                                     start=(k == 0), stop=(k == 2))
                seg = slice(n * CS, (n + 1) * CS)
                if n == 0:
                    if has_bag:
                        nc.vector.tensor_add(pt[:], pt[:], bada_bc[:, seg])
                    nc.scalar.activation(tbl_sb[:, seg], pt[:], AF.Sigmoid)
                elif n == 1:
                    nc.vector.tensor_copy(tbl_sb[:, seg], pt[:])
                else:
                    if has_btg and btg_const is None:
                        nc.vector.tensor_add(pt[:], pt[:], bada_bc[:, seg])
                        nc.scalar.activation(tbl_sb[:, seg], pt[:], AF.Sigmoid)
                    elif btg_const:
                        nc.scalar.activation(tbl_sb[:, seg], pt[:], AF.Sigmoid,
                                             bias=btg_t[:])
                    else:
                        nc.scalar.activation(tbl_sb[:, seg], pt[:], AF.Sigmoid)
            nc.gpsimd.dma_start(tbl[r * 128:(r + 1) * 128, :], tbl_sb[:])

    # ============ B: LN1 + projections (Q/K/V/G) ============
    bd_stack = ExitStack()
    mid = bd_stack.enter_context(tc.tile_pool(name="mid", bufs=1))
    qf = mid.tile([128, 4, NT], BF16)
    kf = mid.tile([128, 4, NT], BF16)
    qf2 = mid.tile([64, 4, NT], BF16)
    kf2 = mid.tile([64, 4, NT], BF16)
    vtm = mid.tile([128, RT, CS], BF16)
    gsig = mid.tile([128, RT, CS], BF16)

    with tc.tile_pool(name="bx", bufs=2) as bx, \
         tc.tile_pool(name="bxw", bufs=1) as bxw, \
         tc.tile_pool(name="ps_tp", bufs=2, space="PSUM") as ps_tp, \
         tc.tile_pool(name="ps_pt", bufs=3, space="PSUM") as ps_pt:
        xnT = bxw.tile([128, 4, NT], BF16)
        mvb, rstdb = _batch_stats(nc, sb, lambda r: re_sb[:, r, :], RT, "bb")
        for r in range(RT):
            xn = bx.tile([128, CS], BF16, tag="xn")
            nc.vector.tensor_scalar(out=xn[:], in0=re_sb[:, r, :],
                                    scalar1=mvb[:, r, 0:1],
                                    scalar2=rstdb[:, r:r + 1],
                                    op0=ALU.subtract, op1=ALU.mult)
            tp = ps_tp.tile([128, CS], BF16, tag="tp")
            for c in range(4):
                nc.tensor.transpose(tp[:, c * 128:(c + 1) * 128],
                                    xn[:, c * 128:(c + 1) * 128], ident[:])
            nc.vector.tensor_copy(
                xnT[:, :, r * 128:(r + 1) * 128],
                tp[:].rearrange("p (k c) -> p k c", k=4))

        # ---- Q/K projections ----
        for (w, bias_sb, has_b, dst) in ((wq, bq_sb, has_bq, qf),
                                         (wk, bk_sb, has_bk, kf)):
            for m in range(4):
                for n in range(2):
                    pt = ps_pt.tile([128, CS], F32, tag="pt")
                    for k in range(4):
                        nc.tensor.matmul(pt[:], w[:, k, m * 128:(m + 1) * 128],
                                         xnT[:, k, n * 512:(n + 1) * 512],
                                         start=(k == 0), stop=(k == 3))
                    dseg = dst[:, m, n * 512:(n + 1) * 512]
                    if has_b:
                        nc.vector.tensor_scalar_add(out=dseg, in0=pt[:],
                                                    scalar1=bias_sb[:, m:m + 1])
                    else:
                        nc.vector.tensor_copy(dseg, pt[:])
        # odd heads' q/k rows duplicated at partition base 0 so every QK
        # matmul issues from PE row-group 0 (mixed row-groups draining into
        # one PSUM bank concurrently crash the device)
        nc.sync.dma_start(qf2[:], qf[64:128, :, :])
        nc.sync.dma_start(kf2[:], kf[64:128, :, :])

        # ---- V (2-way column-tiled: block pair per PSUM tile) ----
        for gpair in range(RT):
            ptv = ps_pt.tile([128, CS], F32, tag="pt")
            for gg in range(2):
                g = 2 * gpair + gg
                for k in range(4):
                    nc.tensor.matmul(ptv[gg * 64:gg * 64 + 64, :],
                                     xnT[:, k, g * 64:(g + 1) * 64],
                                     wv[:, k, :], start=(k == 0), stop=(k == 3),
                                     tile_position=(0, gg * 64))
            nc.scalar.copy(vtm[:, gpair, :], ptv[:])
        # ---- G (sigmoid gate) ----
        for r in range(RT):
            ptg = ps_pt.tile([128, CS], F32, tag="pt")
            for k in range(4):
                nc.tensor.matmul(ptg[:], xnT[:, k, r * 128:(r + 1) * 128],
                                 wg[:, k, :], start=(k == 0), stop=(k == 3))
            nc.scalar.activation(gsig[:, r, :], ptg[:], AF.Sigmoid)

    # ============ C: bias path (2 chunks of 8 blocks) + D: attention ============
    cdp = bd_stack.enter_context(tc.tile_pool(name="cdp", bufs=2))
    ztp = bd_stack.enter_context(tc.tile_pool(name="ztp", bufs=3))
    z2p = bd_stack.enter_context(tc.tile_pool(name="z2p", bufs=1))

    def emit_c_chunk(ch, ps_ze, ps_zo):
        Pall = cdp.tile([128, 8, 1024], BF16, tag="Pall")
        for gl in range(8):
            g = ch * 8 + gl
            zt_t = ztp.tile([128, BLK * BLK], BF16, tag="zt")
            eng = nc.sync if g < 8 else nc.gpsimd
            eng.dma_start(zt_t[:], t["zT"][g])
            z2 = z2p.tile([128, BLK * BLK], BF16, tag="z2")
            nc.vector.tensor_mul(z2[:], zt_t[:], zt_t[:])
            ze = ps_ze.tile([128, 512], F32, tag="ze")
            zo = ps_zo.tile([128, 512], F32, tag="zo")
            for cg in range(4):
                tpos = (0, 32 * cg)
                rows = slice(32 * cg, 32 * cg + 32)
                ev = slice((2 * cg) * 512, (2 * cg + 1) * 512)
                od = slice((2 * cg + 1) * 512, (2 * cg + 2) * 512)
                nc.tensor.matmul(ze[rows, :], wbs_sb[:, 0:32], zt_t[:, ev],
                                 start=True, stop=False, tile_position=tpos)
                nc.tensor.matmul(zo[rows, :], wbs_sb[:, 0:32], zt_t[:, od],
                                 start=True, stop=False, tile_position=tpos)
                nc.tensor.matmul(ze[rows, :], wbs_sb[:, 32:64], z2[:, ev],
                                 start=False, stop=True, tile_position=tpos)
                nc.tensor.matmul(zo[rows, :], wbs_sb[:, 32:64], z2[:, od],
                                 start=False, stop=True, tile_position=tpos)
            nc.scalar.copy(Pall[:, gl, 0:512], ze[:])
            nc.scalar.copy(Pall[:, gl, 512:1024], zo[:])
        # batched reshape round-trip: P[32cg+m, (strip w)] -> pr_d[m, g, i, j]
        prt = pr_d[:]
        sts = []
        for cg in range(4):
            for p2 in range(2):
                src = Pall[32 * cg:32 * cg + 10, :, p2 * 512:(p2 + 1) * 512]
                dst = bass.AP(
                    tensor=prt.tensor,
                    offset=prt.offset + ch * 32768 + (2 * cg + p2) * 512,
                    ap=[[65536, 10], [4096, 8], [1, 512]])
                sts.append(nc.sync.dma_start(dst, src))
        Pr_sb = cdp.tile([128, 4, 640], BF16, tag="Pr")
        for gl in range(4):
            src = bass.AP(tensor=prt.tensor,
                          offset=prt.offset + ch * 32768 + gl * 8192,
                          ap=[[64, 128], [65536, 10], [1, 64]])
            ld = nc.sync.dma_start(
                Pr_sb[:, gl, :].rearrange("p (m j) -> p m j", m=10), src)
            for st in sts:
                add_dep_helper(ld.ins, st.ins, reason="pr RAW")
        # C5: rstd from mu (m=8) and E[z^2] (m=9)
        msq = sb.tile([128, 4, 64], F32, tag="msq")
        nc.vector.tensor_mul(msq[:], Pr_sb[:, :, 512:576], Pr_sb[:, :, 512:576])
        var = sb.tile([128, 4, 64], F32, tag="var")
        nc.vector.scalar_tensor_tensor(out=var[:], in0=Pr_sb[:, :, 576:640],
                                       scalar=EPS, in1=msq[:],
                                       op0=ALU.add, op1=ALU.subtract)
        rv = sb.tile([128, 4, 64], F32, tag="rvc")
        nc.vector.reciprocal_approx_fast(out=rv[:], in_=var[:])
        rstd_c = cdp.tile([128, 4, 64], F32, tag="rstdc")
        nc.scalar.activation(rstd_c[:], rv[:], AF.Sqrt)
        return Pr_sb, rstd_c

    def emit_d_qk(gp, Pr_sb, rstd_c, ps_sc):
        gl = gp % 4
        pb = sb.tile([128, 8, 64], BF16, tag="pb")
        nc.vector.tensor_mul(pb[:],
                             Pr_sb[:, gl, 0:512].rearrange("p (h j) -> p h j", h=H),
                             _b0(rstd_c[:, gl, :], H, at=1))
        sc = ps_sc.tile([128, CS], F32, tag="scog")
        for h in range(H):
            m = h // 2
            for g2 in range(2):
                g = 2 * gp + g2
                qsl = (qf[0:64, m, g * 64:(g + 1) * 64] if h % 2 == 0
                       else qf2[:, m, g * 64:(g + 1) * 64])
                ksl = (kf[0:64, m, g * 64:(g + 1) * 64] if h % 2 == 0
                       else kf2[:, m, g * 64:(g + 1) * 64])
                nc.tensor.matmul(sc[g2 * 64:g2 * 64 + 64, h * 64:(h + 1) * 64],
                                 qsl, ksl, start=True, stop=True,
                                 tile_position=(0, g2 * 64))
        return pb, sc

    def emit_d_sm(gp, pb, sc):
        a_sb = sb.tile([128, CS], BF16, tag="a_sb")
        nc.vector.tensor_add(a_sb[:].rearrange("p (h j) -> p h j", h=H),
                             sc[:].rearrange("p (h j) -> p h j", h=H), pb[:])
        ax = sb.tile([128, CS], BF16, tag="ax")
        nc.scalar.activation(ax[:], a_sb[:], AF.Exp)
        rs = sb.tile([128, H], F32, tag="rs")
        nc.vector.tensor_reduce(rs[:], ax[:].rearrange("p (h j) -> p h j", h=H),
                                axis=mybir.AxisListType.X, op=ALU.add)
        rcp = sb.tile([128, H], F32, tag="rcp")
        nc.vector.reciprocal_approx_fast(out=rcp[:], in_=rs[:])
        gr = sb.tile([128, H, 64], BF16, tag="gr")
        nc.vector.tensor_mul(gr[:],
                             gsig[:, gp, :].rearrange("p (h j) -> p h j", h=H),
                             _b0(rcp[:], 64))
        return ax, gr

    def emit_d_tail(gp, ax, gr, ps_at_a, ps_at_b, ps_o_a, ps_o_b, ps_sc, ps_w):
        aT_a = ps_at_a.tile([64, CS], BF16, tag="aTa")
        aT_b = ps_at_b.tile([128, CS], BF16, tag="aTb")
        for h in range(H):
            nc.tensor.transpose(aT_a[:, h * 64:(h + 1) * 64],
                                ax[0:64, h * 64:(h + 1) * 64],
                                ident[0:64, 0:64], tile_position=(0, 0))
            nc.tensor.transpose(aT_b[64:128, h * 64:(h + 1) * 64],
                                ax[64:128, h * 64:(h + 1) * 64],
                                ident[64:128, 64:128], tile_position=(64, 64))
        aTs = sb.tile([128, CS], BF16, tag="aTs")
        nc.vector.tensor_copy(aTs[0:64, :], aT_a[:])
        nc.vector.tensor_copy(aTs[64:128, :], aT_b[64:128, :])
        o_a = ps_o_a.tile([64, CS], F32, tag="oa")
        o_b = ps_o_b.tile([128, CS], F32, tag="ob")
        for h in range(H):
            nc.tensor.matmul(o_a[:, h * 64:(h + 1) * 64],
                             aTs[0:64, h * 64:(h + 1) * 64],
                             vtm[0:64, gp, h * 64:(h + 1) * 64],
                             start=True, stop=True, tile_position=(0, 0))
            nc.tensor.matmul(o_b[64:128, h * 64:(h + 1) * 64],
                             aTs[64:128, h * 64:(h + 1) * 64],
                             vtm[64:128, gp, h * 64:(h + 1) * 64],
                             start=True, stop=True, tile_position=(64, 64))
        og = sb.tile([128, CS], BF16, tag="og")
        nc.vector.tensor_mul(og[0:64, :].rearrange("p (h j) -> p h j", h=H),
                             o_a[:].rearrange("p (h j) -> p h j", h=H),
                             gr[0:64, :, :])
        nc.vector.tensor_mul(og[64:128, :].rearrange("p (h j) -> p h j", h=H),
                             o_b[64:128, :].rearrange("p (h j) -> p h j", h=H),
                             gr[64:128, :, :])
        ogT = ps_sc.tile([128, CS], BF16, tag="scog")
        for c in range(4):
            nc.tensor.transpose(ogT[:, c * 128:(c + 1) * 128],
                                og[:, c * 128:(c + 1) * 128], ident[:])
        ogs = sb.tile([128, 4, 128], BF16, tag="ogs")
        nc.vector.tensor_copy(ogs[:].rearrange("p k c -> p (k c)"), ogT[:])
        ptw = ps_w.tile([128, CS], F32, tag="ptw")
        for k in range(4):
            nc.tensor.matmul(ptw[:], ogs[:, k, :], wout[:, k, :],
                             start=(k == 0), stop=(k == 3))
        nc.vector.tensor_add(h_sb[:, gp, :], ptw[:], re_sb[:, gp, :])

    with tc.tile_pool(name="ps_ze", bufs=2, space="PSUM") as ps_ze, \
         tc.tile_pool(name="ps_zo", bufs=2, space="PSUM") as ps_zo:
        PrA, rstdA = emit_c_chunk(0, ps_ze, ps_zo)
        PrB, rstdB = emit_c_chunk(1, ps_ze, ps_zo)

    with tc.tile_pool(name="ps_sc", bufs=2, space="PSUM") as ps_sc, \
         tc.tile_pool(name="ps_at_a", bufs=1, space="PSUM") as ps_at_a, \
         tc.tile_pool(name="ps_at_b", bufs=1, space="PSUM") as ps_at_b, \
         tc.tile_pool(name="ps_o_a", bufs=1, space="PSUM") as ps_o_a, \
         tc.tile_pool(name="ps_o_b", bufs=1, space="PSUM") as ps_o_b, \
         tc.tile_pool(name="ps_w", bufs=2, space="PSUM") as ps_w:
        # software-pipelined: QK of gp+1 is emitted before the tail of gp
        state = {}
        pr_of = lambda gp: (PrA, rstdA) if gp < 4 else (PrB, rstdB)
        pb0, sc0 = emit_d_qk(0, *pr_of(0), ps_sc)
        state[0] = (pb0, sc0)
        sm = {0: emit_d_sm(0, pb0, sc0)}
        for gp in range(RT):
            if gp + 1 < RT:
                pbn, scn = emit_d_qk(gp + 1, *pr_of(gp + 1), ps_sc)
                state[gp + 1] = (pbn, scn)
            ax, gr = sm[gp]
            emit_d_tail(gp, ax, gr, ps_at_a, ps_at_b, ps_o_a, ps_o_b,
                        ps_sc, ps_w)
            if gp + 1 < RT:
                sm[gp + 1] = emit_d_sm(gp + 1, *state[gp + 1])

    bd_stack.close()   # free B..D SBUF (qf/kf/vtm/zt rings/Pall/...) before E

    # ================= gathers (issued early on gpsimd queue) =================
    for r in range(RT):
        nc.gpsimd.dma_gather(
            out_ap=gth[:, r:r + 1, :], in_ap=tbl[:],
            idxs_ap=idx_sb[:, r * 8:(r + 1) * 8],
            num_idxs=128, num_idxs_reg=128, elem_size=3 * CS)

    # ================= E: gather-conditioned transition =================
    with tc.tile_pool(name="ep", bufs=2) as ep, \
         tc.tile_pool(name="epw", bufs=1) as epw, \
         tc.tile_pool(name="ps_tt", bufs=2, space="PSUM") as ps_tt, \
         tc.tile_pool(name="ps_A", bufs=2, space="PSUM") as ps_A, \
         tc.tile_pool(name="ps_B", bufs=2, space="PSUM") as ps_B, \
         tc.tile_pool(name="ps_wb", bufs=2, space="PSUM") as ps_wb:
        w1 = epw.tile([128, 4, 2 * CS], BF16)
        nc.sync.dma_start(w1[:], t["w1"][:])
        w2 = epw.tile([128, 4, 2 * CS], BF16)
        nc.sync.dma_start(w2[:], t["w2"][:])
        wb = epw.tile([128, 8, CS], BF16)
        nc.sync.dma_start(wb[:], t["wb"][:])
        tT = epw.tile([128, 4, NT], BF16)
        bb = epw.tile([128, 8, NT], BF16)

        mve, rstde = _batch_stats(nc, sb, lambda r: h_sb[:, r, :], RT, "ee")
        for r in range(RT):
            t0 = ep.tile([128, CS], BF16, tag="t0")
            nc.vector.tensor_scalar(out=t0[:], in0=h_sb[:, r, :],
                                    scalar1=mve[:, r, 0:1],
                                    scalar2=rstde[:, r:r + 1],
                                    op0=ALU.subtract, op1=ALU.mult)
            t1 = ep.tile([128, CS], BF16, tag="t1")
            nc.vector.tensor_mul(t1[:], t0[:], gth[:, r, 0:CS])
            t2 = ep.tile([128, CS], BF16, tag="t2")
            nc.vector.tensor_add(t2[:], t1[:], gth[:, r, CS:2 * CS])
            tp = ps_tt.tile([128, CS], BF16, tag="tt")
            for c in range(4):
                nc.tensor.transpose(tp[:, c * 128:(c + 1) * 128],
                                    t2[:, c * 128:(c + 1) * 128], ident[:])
            nc.vector.tensor_copy(
                tT[:, :, r * 128:(r + 1) * 128],
                tp[:].rearrange("p (k c) -> p k c", k=4))

        for n in range(2):
            for m in range(8):
                pA = ps_A.tile([128, CS], F32, tag="pA")
                for k in range(4):
                    nc.tensor.matmul(pA[:], w1[:, k, m * 128:(m + 1) * 128],
                                     tT[:, k, n * 512:(n + 1) * 512],
                                     start=(k == 0), stop=(k == 3))
                pB = ps_B.tile([128, CS], F32, tag="pB")
                for k in range(4):
                    nc.tensor.matmul(pB[:], w2[:, k, m * 128:(m + 1) * 128],
                                     tT[:, k, n * 512:(n + 1) * 512],
                                     start=(k == 0), stop=(k == 3))
                u1s = ep.tile([128, 512], BF16, tag="u1s")
                nc.scalar.activation(u1s[:], pA[:], AF.Sigmoid)
                u1 = ep.tile([128, 512], F32, tag="u1")
                nc.vector.tensor_mul(u1[:], u1s[:], pA[:])
                nc.vector.tensor_mul(bb[:, m, n * 512:(n + 1) * 512], u1[:], pB[:])
            for r in range(n * 4, n * 4 + 4):
                ptb = ps_wb.tile([128, CS], F32, tag="ptb")
                for k in range(8):
                    nc.tensor.matmul(ptb[:], bb[:, k, r * 128:(r + 1) * 128],
                                     wb[:, k, :], start=(k == 0), stop=(k == 7))
                tr = ep.tile([128, CS], F32, tag="tr")
                nc.vector.tensor_mul(tr[:], ptb[:], gth[:, r, 2 * CS:3 * CS])
                out_t = ep.tile([128, CS], F32, tag="out_t")
                nc.vector.tensor_add(out_t[:], tr[:], h_sb[:, r, :])
                nc.sync.dma_start(t["out"][r * 128:(r + 1) * 128, :], out_t[:])


def build(flags):
    key = ("v2", flags)
    if key in _CACHE:
        return _CACHE[key]
    nc = bacc.Bacc("TRN2", target_bir_lowering=False, debug=False)
    t = _declare(nc)
    with tile.TileContext(nc) as tc:
        with ExitStack() as ctx:
            _emit(ctx, tc, t, flags)
    nc.compile()
    _CACHE[key] = nc
    return nc


def prep_core_inputs(inputs, core):
    """Host-side slicing + weight folding for one core."""
    b = core // 4
    g0 = (core % 4) * NBLK
    r0 = g0 * BLK

    f = lambda k: np.asarray(inputs[k], np.float32)
    ln_w, ln_b = f("ln_w"), f("ln_b")
    sc = 1.0 / np.sqrt(CH)

    def fold(w, scale=1.0):
        return ln_w[:, None] * np.asarray(w, np.float32) * scale

    def foldb(w, scale=1.0):
        return (ln_b @ np.asarray(w, np.float32)) * scale

    Wkv = f("Wkv")
    wq_h, bq_h = fold(inputs["Wq"], sc), foldb(inputs["Wq"], sc)
    wk_h, bk_h = fold(Wkv[:, :CS]), foldb(Wkv[:, :CS])
    wv_h, bv_h = fold(Wkv[:, CS:]), foldb(Wkv[:, CS:])
    wg_h, bg_h = fold(inputs["Wgate"]), foldb(inputs["Wgate"])
    if np.any(bv_h) or np.any(bg_h):
        raise NotImplementedError("nonzero folded v/gate bias unsupported")

    cw = f("adaln_cond_w")
    wada_h = np.concatenate(
        [cw[:, None] * f("W_ada_gate"), cw[:, None] * f("W_ada_bias"),
         cw[:, None] * f("W_tgate")], axis=1)
    bada_h = np.concatenate(
        [f("b_ada_gate"), np.zeros(CS, np.float32), f("b_tgate")]).astype(np.float32)

    # wbs': fold the mean-correction into the weights (bias = P'*rstd);
    # col 8 of the z-pass = mean, col 32+9 of the z^2 pass = E[z^2]
    wbias = f("bias_ln_w")[:, None] * f("Wbias")      # [128, 8]
    wbs_h = np.zeros((CZ, 64), np.float32)
    wbs_h[:, :H] = wbias - wbias.sum(0, keepdims=True) / CZ
    wbs_h[:, 8] = 1.0 / CZ
    wbs_h[:, 32 + 9] = 1.0 / CZ

    def ktile(w, kt):
        w = np.asarray(w, np.float32)
        return np.ascontiguousarray(
            w.reshape(kt, 128, w.shape[1]).transpose(1, 0, 2)).astype(BF)

    # framepair: [16, 64, 64, 128] -> [16, 128, 4096] bf16
    fp = np.asarray(inputs["framepair_embed"][b, g0:g0 + NBLK], np.float32)
    zT = np.ascontiguousarray(
        fp.reshape(NBLK, BLK * BLK, CZ).transpose(0, 2, 1)).astype(BF)

    idx = np.asarray(inputs["rigids_to_res_idx"][b, r0:r0 + NT]).astype(np.int16)
    idx_w = np.empty((128, NT // 16), np.int16)
    for p in range(16):
        idx_w[p] = idx[p::16]
    idx_w[16:] = np.tile(idx_w[:16], (7, 1))

    btg = f("b_tgate")
    btg_const = float(btg[0]) if np.all(btg == btg[0]) else None
    has_btg = bool(np.any(btg))

    return {
        "re": np.ascontiguousarray(inputs["rigids_embed"][b, r0:r0 + NT]).astype(BF),
        "zT": zT,
        "s": np.ascontiguousarray(inputs["s"][b]).astype(np.float32),
        "idx": idx_w,
        "wq": ktile(wq_h, 4), "wk": ktile(wk_h, 4), "wv": ktile(wv_h, 4),
        "wg": ktile(wg_h, 4), "wout": ktile(inputs["Wout"], 4),
        "w1": ktile(inputs["W1"], 4), "w2": ktile(inputs["W2"], 4),
        "wb": ktile(inputs["Wb"], 8), "wada": ktile(wada_h, 3),
        "wbs": wbs_h.astype(BF),
        "bq": np.ascontiguousarray(bq_h.reshape(4, 128).T),
        "bk": np.ascontiguousarray(bk_h.reshape(4, 128).T),
        "bada": bada_h,
    }, (bool(np.any(bq_h)), bool(np.any(bk_h)), bool(np.any(f("b_ada_gate"))),
        btg_const, has_btg)


def kernel(**inputs):
    mask = np.asarray(inputs["rigids_mask"])
    if not np.all(mask == 1.0):
        print("WARNING: rigids_mask not all ones; kernel assumes ones", file=sys.stderr)

    in_maps, flags = [], None
    for core in range(NCORES):
        m, flags = prep_core_inputs(inputs, core)
        in_maps.append(m)

    nc = build(flags)
    res = run_bass_kernel_spmd(nc, in_maps, core_ids=list(range(NCORES)))

    out = np.empty((B, N, CS), np.float32)
    for core in range(NCORES):
        b = core // 4
        r0 = (core % 4) * NT
        out[b, r0:r0 + NT] = res.results[core]["out"]
    return out


# revision 16
# speedup vs baseline: 1.8960x; 1.0593x over previous
"""BlockTransformerPairBias Trainium2 kernel (v2 — pipelined).

Sharding: 8 cores = (batch 0/1) x (4 groups of 16 attention blocks).
Each core computes its 1024 tokens end-to-end; no collectives.

v2 restructure vs baseline:
- PSUM/SBUF double buffering everywhere (no bufs=1 serialization)
- batched LN statistics; ACT pinned to one transcendental per phase
  (table reloads cost 46us in the baseline); PSUM->SBUF copies ride
  ACT's Copy path (no table) or DVE
- bias-path weights pre-folded (wbs' = w - S/128) so the pair bias is
  just P'*rstd; the P reshape round-trip is batched per 8-block chunk
- attention software-pipelined across block-pairs (QK of gp+1 emitted
  before the tail of gp); V projection 2-way column-tiled
- re/h kept in bf16; zT loads split across sync+gpsimd DMA queues
"""

import sys

sys.path.insert(0, "/opt/trn_rl_repo")

from contextlib import ExitStack

import numpy as np
import ml_dtypes

import concourse.bass as bass
import concourse.tile as tile
from concourse import bacc, mybir
from concourse.bass_utils import run_bass_kernel_spmd
from concourse.masks import make_identity
from concourse.tile import add_dep_helper

F32 = mybir.dt.float32
BF16 = mybir.dt.bfloat16
I16 = mybir.dt.int16
AF = mybir.ActivationFunctionType
ALU = mybir.AluOpType
BF = ml_dtypes.bfloat16

B, N, NRES = 2, 4096, 1024
CS, CC, CZ, H, BLK = 512, 384, 128, 8, 64
CH = CS // H          # 64
NB = N // BLK         # 64
NCORES = 8
NBLK = NB * B // NCORES   # 16 blocks per core
NT = NBLK * BLK           # 1024 tokens per core
RT = NT // 128            # 8 token tiles
EPS = 1e-5

_CACHE = {}


def _declare(nc):
    t = {}

    def inp(name, shape, dt):
        t[name] = nc.dram_tensor(name, list(shape), dt, kind="ExternalInput").ap()

    inp("re", (NT, CS), BF16)
    inp("zT", (NBLK, CZ, BLK * BLK), BF16)
    inp("s", (NRES, CC), F32)
    inp("idx", (128, NT // 16), I16)
    inp("wq", (128, 4, CS), BF16)
    inp("wk", (128, 4, CS), BF16)
    inp("wv", (128, 4, CS), BF16)
    inp("wg", (128, 4, CS), BF16)
    inp("wout", (128, 4, CS), BF16)
    inp("w1", (128, 4, 2 * CS), BF16)
    inp("w2", (128, 4, 2 * CS), BF16)
    inp("wb", (128, 8, CS), BF16)
    inp("wada", (128, 3, 3 * CS), BF16)
    inp("wbs", (CZ, 64), BF16)
    inp("bq", (128, 4), F32)
    inp("bk", (128, 4), F32)
    inp("bada", (3 * CS,), F32)
    t["out"] = nc.dram_tensor("out", [NT, CS], F32, kind="ExternalOutput").ap()
    return t


def _bcast(ap, p=128):
    """Broadcast a 1-D DRAM AP across p partitions."""
    return bass.AP(tensor=ap.tensor, offset=ap.offset, ap=[[0, p]] + list(ap.ap))


def _b0(ap_, reps, at=None):
    """Insert a 0-stride broadcast dim into a free position of an AP."""
    lst = list(ap_.ap)
    pos = len(lst) if at is None else at
    lst.insert(pos, [0, reps])
    return bass.AP(tensor=ap_.tensor, offset=ap_.offset, ap=lst)


def _batch_stats(nc, sb, src_rc, nrt, tagp):
    """LN stats for nrt row-tiles: returns (mv [128,nrt,2], rstd [128,nrt]).

    One ACT Sqrt for the whole batch (rstd = sqrt(1/(var+eps)) via DVE
    reciprocal_approx_fast), so the ACT table loads once per phase.
    src_rc(r) must return the [128, C] AP of tile r.
    """
    st6 = sb.tile([128, nrt, 6], F32, tag=tagp + "st6")
    for r in range(nrt):
        nc.vector.bn_stats(st6[:, r, :], src_rc(r))
    mv = sb.tile([128, nrt, 2], F32, tag=tagp + "mv")
    for r in range(nrt):
        nc.vector.bn_aggr(mv[:, r, :], st6[:, r, :])
    vc = sb.tile([128, nrt], F32, tag=tagp + "vc")
    nc.vector.tensor_scalar_add(
        vc[:], mv[:, :, 1:2].rearrange("p r o -> p (r o)"), EPS)
    rv = sb.tile([128, nrt], F32, tag=tagp + "rv")
    nc.vector.reciprocal_approx_fast(out=rv[:], in_=vc[:])
    rstd = sb.tile([128, nrt], F32, tag=tagp + "rstd")
    nc.scalar.activation(rstd[:], rv[:], AF.Sqrt)
    return mv, rstd


def _emit(ctx, tc, t, flags):
    nc = tc.nc
    has_bq, has_bk, has_bag, btg_const, has_btg = flags

    consts = ctx.enter_context(tc.tile_pool(name="consts", bufs=1))
    top = ctx.enter_context(tc.tile_pool(name="top", bufs=1))
    sb = ctx.enter_context(tc.tile_pool(name="sb", bufs=2))
    dramp = ctx.enter_context(tc.tile_pool(name="dram", bufs=1, space="DRAM"))

    # ---- early DMAs: re first (B stats start immediately), then weights ----
    ident = consts.tile([128, 128], BF16)
    make_identity(nc, ident[:])
    re_sb = top.tile([128, RT, CS], BF16)
    nc.sync.dma_start(re_sb[:], t["re"].rearrange("(r p) c -> p r c", p=128))
    idx_sb = consts.tile([128, NT // 16], I16)
    nc.sync.dma_start(idx_sb[:], t["idx"][:])
    wbs_sb = consts.tile([CZ, 64], BF16)
    nc.sync.dma_start(wbs_sb[:], t["wbs"][:])
    bq_sb = consts.tile([128, 4], F32)
    bk_sb = consts.tile([128, 4], F32)
    if has_bq:
        nc.sync.dma_start(bq_sb[:], t["bq"][:])
    if has_bk:
        nc.sync.dma_start(bk_sb[:], t["bk"][:])
    wq = consts.tile([128, 4, CS], BF16)
    nc.sync.dma_start(wq[:], t["wq"][:])
    wk = consts.tile([128, 4, CS], BF16)
    nc.sync.dma_start(wk[:], t["wk"][:])
    wv = consts.tile([128, 4, CS], BF16)
    nc.sync.dma_start(wv[:], t["wv"][:])
    wg = consts.tile([128, 4, CS], BF16)
    nc.sync.dma_start(wg[:], t["wg"][:])
    wout = consts.tile([128, 4, CS], BF16)
    nc.sync.dma_start(wout[:], t["wout"][:])
    btg_t = consts.tile([128, 1], F32)
    if btg_const:
        nc.vector.memset(btg_t[:], btg_const)

    h_sb = top.tile([128, RT, CS], BF16)
    gth = top.tile([128, RT, 3 * CS], BF16)

    tbl = dramp.tile([NRES, 3 * CS], BF16)
    pr_d = dramp.tile([10, NBLK * BLK * BLK], BF16)

    bd_stack = ExitStack()
    mid = bd_stack.enter_context(tc.tile_pool(name="mid", bufs=1))
    qf = mid.tile([128, 4, NT], BF16)
    kf = mid.tile([128, 4, NT], BF16)
    qf2 = mid.tile([64, 4, NT], BF16)
    kf2 = mid.tile([64, 4, NT], BF16)
    vtm = mid.tile([128, RT, CS], BF16)
    gsig = mid.tile([128, RT, CS], BF16)
    cdp = bd_stack.enter_context(tc.tile_pool(name="cdp", bufs=2))
    bc_stack = ExitStack()
    xnp = bc_stack.enter_context(tc.tile_pool(name="xnp", bufs=1))
    xnT = xnp.tile([128, 4, NT], BF16)

    # ---- B stats + xnT first: PE gets work within ~6us ----
    with tc.tile_pool(name="bx", bufs=2) as bx, \
         tc.tile_pool(name="ps_tp", bufs=2, space="PSUM") as ps_tp:
        mvb, rstdb = _batch_stats(nc, sb, lambda r: re_sb[:, r, :], RT, "bb")
        for r in range(RT):
            xn = bx.tile([128, CS], BF16, tag="xn")
            nc.vector.tensor_scalar(out=xn[:], in0=re_sb[:, r, :],
                                    scalar1=mvb[:, r, 0:1],
                                    scalar2=rstdb[:, r:r + 1],
                                    op0=ALU.subtract, op1=ALU.mult)
            tp = ps_tp.tile([128, CS], BF16, tag="tp")
            for c in range(4):
                nc.tensor.transpose(tp[:, c * 128:(c + 1) * 128],
                                    xn[:, c * 128:(c + 1) * 128], ident[:])
            nc.vector.tensor_copy(
                xnT[:, :, r * 128:(r + 1) * 128],
                tp[:].rearrange("p (k c) -> p k c", k=4))

    # ================= P1: cond tables =================
    with tc.tile_pool(name="p1", bufs=2) as p1p, \
         tc.tile_pool(name="p1w", bufs=1) as p1w, \
         tc.tile_pool(name="ps_tp1", bufs=2, space="PSUM") as ps_tp1, \
         tc.tile_pool(name="ps_p1", bufs=3, space="PSUM") as ps_p1:
        wada = p1w.tile([128, 3, 3 * CS], BF16)
        nc.sync.dma_start(wada[:], t["wada"][:])
        bada_bc = p1w.tile([128, 3 * CS], F32)
        if has_bag or (has_btg and btg_const is None):
            nc.sync.dma_start(bada_bc[:], _bcast(t["bada"]))
        s_all = p1w.tile([128, NRES // 128, CC], F32)
        nc.gpsimd.dma_start(s_all[:], t["s"].rearrange("(r p) c -> p r c", p=128))

        mv1, rstd1 = _batch_stats(nc, sb, lambda r: s_all[:, r, :], 8, "p1")
        for r in range(NRES // 128):
            cond = p1p.tile([128, CC], BF16, tag="cond")
            nc.vector.tensor_scalar(out=cond[:], in0=s_all[:, r, :],
                                    scalar1=mv1[:, r, 0:1],
                                    scalar2=rstd1[:, r:r + 1],
                                    op0=ALU.subtract, op1=ALU.mult)
            tp = ps_tp1.tile([128, CC], BF16, tag="tp1")
            for c in range(3):
                nc.tensor.transpose(tp[:, c * 128:(c + 1) * 128],
                                    cond[:, c * 128:(c + 1) * 128], ident[:])
            ct = p1p.tile([128, 3, 128], BF16, tag="ct")
            nc.vector.tensor_copy(ct[:].rearrange("p k c -> p (k c)"), tp[:])
            tbl_sb = p1p.tile([128, 3 * CS], BF16, tag="tblsb")
            for n in range(3):
                pt = ps_p1.tile([128, CS], F32, tag="p1pt")
                for k in range(3):
                    nc.tensor.matmul(pt[:], ct[:, k, :],
                                     wada[:, k, n * CS:(n + 1) * CS],
                                     start=(k == 0), stop=(k == 2))
                seg = slice(n * CS, (n + 1) * CS)
                if n == 0:
                    if has_bag:
                        nc.vector.tensor_add(pt[:], pt[:], bada_bc[:, seg])
                    nc.scalar.activation(tbl_sb[:, seg], pt[:], AF.Sigmoid)
                elif n == 1:
                    nc.vector.tensor_copy(tbl_sb[:, seg], pt[:])
                else:
                    if has_btg and btg_const is None:
                        nc.vector.tensor_add(pt[:], pt[:], bada_bc[:, seg])
                        nc.scalar.activation(tbl_sb[:, seg], pt[:], AF.Sigmoid)
                    elif btg_const:
                        nc.scalar.activation(tbl_sb[:, seg], pt[:], AF.Sigmoid,
                                             bias=btg_t[:])
                    else:
                        nc.scalar.activation(tbl_sb[:, seg], pt[:], AF.Sigmoid)
            nc.gpsimd.dma_start(tbl[r * 128:(r + 1) * 128, :], tbl_sb[:])

    # gathers right after P1 on the gpsimd queue (consumed in E)
    for r in range(RT):
        nc.gpsimd.dma_gather(
            out_ap=gth[:, r:r + 1, :], in_ap=tbl[:],
            idxs_ap=idx_sb[:, r * 8:(r + 1) * 8],
            num_idxs=128, num_idxs_reg=128, elem_size=3 * CS)

    # ====== B projections interleaved with C bias-path blocks ======
    pallp = bc_stack.enter_context(tc.tile_pool(name="pallp", bufs=2))
    ztp = bc_stack.enter_context(tc.tile_pool(name="ztp", bufs=2))
    z2p = bc_stack.enter_context(tc.tile_pool(name="z2p", bufs=1))

    ps_pt = bc_stack.enter_context(tc.tile_pool(name="ps_pt", bufs=3, space="PSUM"))
    ps_ze = bc_stack.enter_context(tc.tile_pool(name="ps_ze", bufs=2, space="PSUM"))
    ps_zo = bc_stack.enter_context(tc.tile_pool(name="ps_zo", bufs=2, space="PSUM"))

    Pall = {0: None, 1: None}

    def emit_c_block(g):
        ch, gl = g // 8, g % 8
        if gl == 0:
            Pall[ch] = pallp.tile([128, 8, 1024], BF16, tag="Pall", name="Pall")
        zt_t = ztp.tile([128, BLK * BLK], BF16, tag="zt")
        nc.sync.dma_start(zt_t[:], t["zT"][g])
        z2 = z2p.tile([128, BLK * BLK], BF16, tag="z2")
        nc.vector.tensor_mul(z2[:], zt_t[:], zt_t[:])
        ze = ps_ze.tile([128, 512], F32, tag="ze")
        zo = ps_zo.tile([128, 512], F32, tag="zo")
        for cg in range(4):
            tpos = (0, 32 * cg)
            rows = slice(32 * cg, 32 * cg + 32)
            ev = slice((2 * cg) * 512, (2 * cg + 1) * 512)
            od = slice((2 * cg + 1) * 512, (2 * cg + 2) * 512)
            nc.tensor.matmul(ze[rows, :], wbs_sb[:, 0:32], zt_t[:, ev],
                             start=True, stop=False, tile_position=tpos)
            nc.tensor.matmul(zo[rows, :], wbs_sb[:, 0:32], zt_t[:, od],
                             start=True, stop=False, tile_position=tpos)
            nc.tensor.matmul(ze[rows, :], wbs_sb[:, 32:64], z2[:, ev],
                             start=False, stop=True, tile_position=tpos)
            nc.tensor.matmul(zo[rows, :], wbs_sb[:, 32:64], z2[:, od],
                             start=False, stop=True, tile_position=tpos)
        nc.scalar.copy(Pall[ch][:, gl, 0:512], ze[:])
        nc.scalar.copy(Pall[ch][:, gl, 512:1024], zo[:])

    def emit_c_roundtrip(ch):
        # reshape round-trip P[32cg+m, (strip w)] -> pr_d[m, g, i, j];
        # DMAs ride the scalar queue right behind the Pall copies
        prt = pr_d[:]
        sts = []
        for cg in range(4):
            for p2 in range(2):
                src = Pall[ch][32 * cg:32 * cg + 10, :, p2 * 512:(p2 + 1) * 512]
                dst = bass.AP(
                    tensor=prt.tensor,
                    offset=prt.offset + ch * 32768 + (2 * cg + p2) * 512,
                    ap=[[65536, 10], [4096, 8], [1, 512]])
                sts.append(nc.scalar.dma_start(dst, src))
        Pr_sb = cdp.tile([128, 4, 640], BF16, tag="Pr")
        for gl in range(4):
            src = bass.AP(tensor=prt.tensor,
                          offset=prt.offset + ch * 32768 + gl * 8192,
                          ap=[[64, 128], [65536, 10], [1, 64]])
            ld = nc.scalar.dma_start(
                Pr_sb[:, gl, :].rearrange("p (m j) -> p m j", m=10), src)
            for st in sts:
                add_dep_helper(ld.ins, st.ins, reason="pr RAW")
        msq = sb.tile([128, 4, 64], F32, tag="msq")
        nc.vector.tensor_mul(msq[:], Pr_sb[:, :, 512:576], Pr_sb[:, :, 512:576])
        var = sb.tile([128, 4, 64], F32, tag="var")
        nc.vector.scalar_tensor_tensor(out=var[:], in0=Pr_sb[:, :, 576:640],
                                       scalar=EPS, in1=msq[:],
                                       op0=ALU.add, op1=ALU.subtract)
        rv = sb.tile([128, 4, 64], F32, tag="rvc")
        nc.vector.reciprocal_approx_fast(out=rv[:], in_=var[:])
        rstd_c = cdp.tile([128, 4, 64], F32, tag="rstdc")
        nc.scalar.activation(rstd_c[:], rv[:], AF.Sqrt)
        return Pr_sb, rstd_c

    # Q
    for m in range(4):
        for n in range(2):
            pt = ps_pt.tile([128, CS], F32, tag="pt")
            for k in range(4):
                nc.tensor.matmul(pt[:], wq[:, k, m * 128:(m + 1) * 128],
                                 xnT[:, k, n * 512:(n + 1) * 512],
                                 start=(k == 0), stop=(k == 3))
            dseg = qf[:, m, n * 512:(n + 1) * 512]
            if has_bq:
                nc.vector.tensor_scalar_add(out=dseg, in0=pt[:],
                                            scalar1=bq_sb[:, m:m + 1])
            else:
                nc.vector.tensor_copy(dseg, pt[:])
    for g in range(0, 3):
        emit_c_block(g)
    # K
    for m in range(4):
        for n in range(2):
            pt = ps_pt.tile([128, CS], F32, tag="pt")
            for k in range(4):
                nc.tensor.matmul(pt[:], wk[:, k, m * 128:(m + 1) * 128],
                                 xnT[:, k, n * 512:(n + 1) * 512],
                                 start=(k == 0), stop=(k == 3))
            dseg = kf[:, m, n * 512:(n + 1) * 512]
            if has_bk:
                nc.vector.tensor_scalar_add(out=dseg, in0=pt[:],
                                            scalar1=bk_sb[:, m:m + 1])
            else:
                nc.vector.tensor_copy(dseg, pt[:])
    nc.sync.dma_start(qf2[:], qf[64:128, :, :])
    nc.sync.dma_start(kf2[:], kf[64:128, :, :])
    for g in range(3, 8):
        emit_c_block(g)
    PrA, rstdA = emit_c_roundtrip(0)
    # V (2-way column-tiled: block pair per PSUM tile)
    for gpair in range(RT):
        ptv = ps_pt.tile([128, CS], F32, tag="pt")
        for gg in range(2):
            g = 2 * gpair + gg
            for k in range(4):
                nc.tensor.matmul(ptv[gg * 64:gg * 64 + 64, :],
                                 xnT[:, k, g * 64:(g + 1) * 64],
                                 wv[:, k, :], start=(k == 0), stop=(k == 3),
                                 tile_position=(0, gg * 64))
        nc.scalar.copy(vtm[:, gpair, :], ptv[:])
    for g in range(8, 12):
        emit_c_block(g)
    # G
    for r in range(RT):
        ptg = ps_pt.tile([128, CS], F32, tag="pt")
        for k in range(4):
            nc.tensor.matmul(ptg[:], xnT[:, k, r * 128:(r + 1) * 128],
                             wg[:, k, :], start=(k == 0), stop=(k == 3))
        nc.scalar.activation(gsig[:, r, :], ptg[:], AF.Sigmoid)
    for g in range(12, 16):
        emit_c_block(g)
    PrB, rstdB = emit_c_roundtrip(1)
    bc_stack.close()

    # ================= D1: QK + softmax + transposed A =================
    dmid = bd_stack.enter_context(tc.tile_pool(name="dmid", bufs=1))
    aTs_all = dmid.tile([128, RT, CS], BF16)
    gr_all = dmid.tile([128, RT, CS], BF16)
    pr_of = lambda gp: (PrA, rstdA) if gp < 4 else (PrB, rstdB)

    def emit_d1_qk(gp, ps_sc):
        Pr_sb, rstd_c = pr_of(gp)
        gl = gp % 4
        pb = sb.tile([128, 8, 64], BF16, tag="pb")
        nc.vector.tensor_mul(pb[:],
                             Pr_sb[:, gl, 0:512].rearrange("p (h j) -> p h j", h=H),
                             _b0(rstd_c[:, gl, :], H, at=1))
        scp = ps_sc.tile([128, CS], F32, tag="sc")
        for h in range(H):
            m = h // 2
            for g2 in range(2):
                g = 2 * gp + g2
                qsl = (qf[0:64, m, g * 64:(g + 1) * 64] if h % 2 == 0
                       else qf2[:, m, g * 64:(g + 1) * 64])
                ksl = (kf[0:64, m, g * 64:(g + 1) * 64] if h % 2 == 0
                       else kf2[:, m, g * 64:(g + 1) * 64])
                nc.tensor.matmul(scp[g2 * 64:g2 * 64 + 64, h * 64:(h + 1) * 64],
                                 qsl, ksl, start=True, stop=True,
                                 tile_position=(0, g2 * 64))
        return pb, scp

    def emit_d1_rest(gp, pb, scp, ps_at_a, ps_at_b):
        a_sb = sb.tile([128, CS], BF16, tag="a_sb")
        nc.vector.tensor_add(a_sb[:].rearrange("p (h j) -> p h j", h=H),
                             scp[:].rearrange("p (h j) -> p h j", h=H), pb[:])
        ax = sb.tile([128, CS], BF16, tag="ax")
        nc.scalar.activation(ax[:], a_sb[:], AF.Exp)
        rs = sb.tile([128, H], F32, tag="rs")
        nc.vector.tensor_reduce(rs[:], ax[:].rearrange("p (h j) -> p h j", h=H),
                                axis=mybir.AxisListType.X, op=ALU.add)
        rcp = sb.tile([128, H], F32, tag="rcp")
        nc.vector.reciprocal_approx_fast(out=rcp[:], in_=rs[:])
        nc.vector.tensor_mul(gr_all[:, gp, :].rearrange("p (h j) -> p h j", h=H),
                             gsig[:, gp, :].rearrange("p (h j) -> p h j", h=H),
                             _b0(rcp[:], 64))
        aT_a = ps_at_a.tile([64, CS], BF16, tag="aTa")
        aT_b = ps_at_b.tile([128, CS], BF16, tag="aTb")
        for h in range(H):
            nc.tensor.transpose(aT_a[:, h * 64:(h + 1) * 64],
                                ax[0:64, h * 64:(h + 1) * 64],
                                ident[0:64, 0:64], tile_position=(0, 0))
            nc.tensor.transpose(aT_b[64:128, h * 64:(h + 1) * 64],
                                ax[64:128, h * 64:(h + 1) * 64],
                                ident[64:128, 64:128], tile_position=(64, 64))
        nc.scalar.copy(aTs_all[0:64, gp, :], aT_a[:])
        nc.scalar.copy(aTs_all[64:128, gp, :], aT_b[64:128, :])

    with tc.tile_pool(name="ps_sc", bufs=2, space="PSUM") as ps_sc, \
         tc.tile_pool(name="ps_at_a", bufs=2, space="PSUM") as ps_at_a, \
         tc.tile_pool(name="ps_at_b", bufs=2, space="PSUM") as ps_at_b:
        prev = emit_d1_qk(0, ps_sc)
        for gp in range(RT):
            nxt = emit_d1_qk(gp + 1, ps_sc) if gp + 1 < RT else None
            emit_d1_rest(gp, *prev, ps_at_a, ps_at_b)
            prev = nxt

    # ================= D2: AV + gate + Wout + residual =================
    def emit_d2_av(gp, ps_o_a, ps_o_b):
        o_a = ps_o_a.tile([64, CS], F32, tag="oa")
        o_b = ps_o_b.tile([128, CS], F32, tag="ob")
        for h in range(H):
            nc.tensor.matmul(o_a[:, h * 64:(h + 1) * 64],
                             aTs_all[0:64, gp, h * 64:(h + 1) * 64],
                             vtm[0:64, gp, h * 64:(h + 1) * 64],
                             start=True, stop=True, tile_position=(0, 0))
            nc.tensor.matmul(o_b[64:128, h * 64:(h + 1) * 64],
                             aTs_all[64:128, gp, h * 64:(h + 1) * 64],
                             vtm[64:128, gp, h * 64:(h + 1) * 64],
                             start=True, stop=True, tile_position=(64, 64))
        return o_a, o_b

    def emit_d2_rest(gp, o_a, o_b, ps_og, ps_w):
        og = sb.tile([128, CS], BF16, tag="og")
        nc.vector.tensor_mul(og[0:64, :].rearrange("p (h j) -> p h j", h=H),
                             o_a[:].rearrange("p (h j) -> p h j", h=H),
                             gr_all[0:64, gp, :].rearrange("p (h j) -> p h j", h=H))
        nc.vector.tensor_mul(og[64:128, :].rearrange("p (h j) -> p h j", h=H),
                             o_b[64:128, :].rearrange("p (h j) -> p h j", h=H),
                             gr_all[64:128, gp, :].rearrange("p (h j) -> p h j", h=H))
        ogT = ps_og.tile([128, CS], BF16, tag="ogT")
        for c in range(4):
            nc.tensor.transpose(ogT[:, c * 128:(c + 1) * 128],
                                og[:, c * 128:(c + 1) * 128], ident[:])
        ogs = sb.tile([128, 4, 128], BF16, tag="ogs")
        nc.scalar.copy(ogs[:].rearrange("p k c -> p (k c)"), ogT[:])
        ptw = ps_w.tile([128, CS], F32, tag="ptw")
        for k in range(4):
            nc.tensor.matmul(ptw[:], ogs[:, k, :], wout[:, k, :],
                             start=(k == 0), stop=(k == 3))
        nc.vector.tensor_add(h_sb[:, gp, :], ptw[:], re_sb[:, gp, :])

    with tc.tile_pool(name="ps_o_a", bufs=2, space="PSUM") as ps_o_a, \
         tc.tile_pool(name="ps_o_b", bufs=2, space="PSUM") as ps_o_b, \
         tc.tile_pool(name="ps_og", bufs=2, space="PSUM") as ps_og, \
         tc.tile_pool(name="ps_w", bufs=2, space="PSUM") as ps_w:
        prev = emit_d2_av(0, ps_o_a, ps_o_b)
        for gp in range(RT):
            nxt = emit_d2_av(gp + 1, ps_o_a, ps_o_b) if gp + 1 < RT else None
            emit_d2_rest(gp, *prev, ps_og, ps_w)
            prev = nxt

    bd_stack.close()   # free B..D SBUF (qf/kf/vtm/zt rings/Pall/...) before E

    # ================= E: gather-conditioned transition =================
    with tc.tile_pool(name="ep", bufs=2) as ep, \
         tc.tile_pool(name="epw", bufs=1) as epw, \
         tc.tile_pool(name="ps_tt", bufs=1, space="PSUM") as ps_tt, \
         tc.tile_pool(name="ps_A", bufs=3, space="PSUM") as ps_A, \
         tc.tile_pool(name="ps_B", bufs=3, space="PSUM") as ps_B, \
         tc.tile_pool(name="ps_wb", bufs=1, space="PSUM") as ps_wb:
        w1 = epw.tile([128, 4, 2 * CS], BF16)
        nc.sync.dma_start(w1[:], t["w1"][:])
        w2 = epw.tile([128, 4, 2 * CS], BF16)
        nc.sync.dma_start(w2[:], t["w2"][:])
        wb = epw.tile([128, 8, CS], BF16)
        nc.sync.dma_start(wb[:], t["wb"][:])
        tT = epw.tile([128, 4, NT], BF16)
        bb = epw.tile([128, 8, NT], BF16)

        mve, rstde = _batch_stats(nc, sb, lambda r: h_sb[:, r, :], RT, "ee")
        for r in range(RT):
            t0 = ep.tile([128, CS], BF16, tag="t0")
            nc.vector.tensor_scalar(out=t0[:], in0=h_sb[:, r, :],
                                    scalar1=mve[:, r, 0:1],
                                    scalar2=rstde[:, r:r + 1],
                                    op0=ALU.subtract, op1=ALU.mult)
            t1 = ep.tile([128, CS], BF16, tag="t1")
            nc.vector.tensor_mul(t1[:], t0[:], gth[:, r, 0:CS])
            t2 = ep.tile([128, CS], BF16, tag="t2")
            nc.vector.tensor_add(t2[:], t1[:], gth[:, r, CS:2 * CS])
            tp = ps_tt.tile([128, CS], BF16, tag="tt")
            for c in range(4):
                nc.tensor.transpose(tp[:, c * 128:(c + 1) * 128],
                                    t2[:, c * 128:(c + 1) * 128], ident[:])
            nc.vector.tensor_copy(
                tT[:, :, r * 128:(r + 1) * 128],
                tp[:].rearrange("p (k c) -> p k c", k=4))

        for n in range(2):
            for m in range(8):
                pA = ps_A.tile([128, CS], F32, tag="pA")
                for k in range(4):
                    nc.tensor.matmul(pA[:], w1[:, k, m * 128:(m + 1) * 128],
                                     tT[:, k, n * 512:(n + 1) * 512],
                                     start=(k == 0), stop=(k == 3))
                pB = ps_B.tile([128, CS], F32, tag="pB")
                for k in range(4):
                    nc.tensor.matmul(pB[:], w2[:, k, m * 128:(m + 1) * 128],
                                     tT[:, k, n * 512:(n + 1) * 512],
                                     start=(k == 0), stop=(k == 3))
                u1s = ep.tile([128, 512], BF16, tag="u1s")
                nc.scalar.activation(u1s[:], pA[:], AF.Sigmoid)
                u1 = ep.tile([128, 512], F32, tag="u1")
                nc.vector.tensor_mul(u1[:], u1s[:], pA[:])
                nc.vector.tensor_mul(bb[:, m, n * 512:(n + 1) * 512], u1[:], pB[:])
            for r in range(n * 4, n * 4 + 4):
                ptb = ps_wb.tile([128, CS], F32, tag="ptb")
                for k in range(8):
                    nc.tensor.matmul(ptb[:], bb[:, k, r * 128:(r + 1) * 128],
                                     wb[:, k, :], start=(k == 0), stop=(k == 7))
                tr = ep.tile([128, CS], F32, tag="tr")
                nc.vector.tensor_mul(tr[:], ptb[:], gth[:, r, 2 * CS:3 * CS])
                out_t = ep.tile([128, CS], F32, tag="out_t")
                nc.vector.tensor_add(out_t[:], tr[:], h_sb[:, r, :])
                nc.sync.dma_start(t["out"][r * 128:(r + 1) * 128, :], out_t[:])


def build(flags):
    key = ("v2", flags)
    if key in _CACHE:
        return _CACHE[key]
    nc = bacc.Bacc("TRN2", target_bir_lowering=False, debug=False)
    t = _declare(nc)
    with tile.TileContext(nc) as tc:
        with ExitStack() as ctx:
            _emit(ctx, tc, t, flags)
    nc.compile()
    _CACHE[key] = nc
    return nc


def prep_core_inputs(inputs, core):
    """Host-side slicing + weight folding for one core."""
    b = core // 4
    g0 = (core % 4) * NBLK
    r0 = g0 * BLK

    f = lambda k: np.asarray(inputs[k], np.float32)
    ln_w, ln_b = f("ln_w"), f("ln_b")
    sc = 1.0 / np.sqrt(CH)

    def fold(w, scale=1.0):
        return ln_w[:, None] * np.asarray(w, np.float32) * scale

    def foldb(w, scale=1.0):
        return (ln_b @ np.asarray(w, np.float32)) * scale

    Wkv = f("Wkv")
    wq_h, bq_h = fold(inputs["Wq"], sc), foldb(inputs["Wq"], sc)
    wk_h, bk_h = fold(Wkv[:, :CS]), foldb(Wkv[:, :CS])
    wv_h, bv_h = fold(Wkv[:, CS:]), foldb(Wkv[:, CS:])
    wg_h, bg_h = fold(inputs["Wgate"]), foldb(inputs["Wgate"])
    if np.any(bv_h) or np.any(bg_h):
        raise NotImplementedError("nonzero folded v/gate bias unsupported")

    cw = f("adaln_cond_w")
    wada_h = np.concatenate(
        [cw[:, None] * f("W_ada_gate"), cw[:, None] * f("W_ada_bias"),
         cw[:, None] * f("W_tgate")], axis=1)
    bada_h = np.concatenate(
        [f("b_ada_gate"), np.zeros(CS, np.float32), f("b_tgate")]).astype(np.float32)

    # wbs': fold the mean-correction into the weights (bias = P'*rstd);
    # col 8 of the z-pass = mean, col 32+9 of the z^2 pass = E[z^2]
    wbias = f("bias_ln_w")[:, None] * f("Wbias")      # [128, 8]
    wbs_h = np.zeros((CZ, 64), np.float32)
    wbs_h[:, :H] = wbias - wbias.sum(0, keepdims=True) / CZ
    wbs_h[:, 8] = 1.0 / CZ
    wbs_h[:, 32 + 9] = 1.0 / CZ

    def ktile(w, kt):
        w = np.asarray(w, np.float32)
        return np.ascontiguousarray(
            w.reshape(kt, 128, w.shape[1]).transpose(1, 0, 2)).astype(BF)

    # framepair: [16, 64, 64, 128] -> [16, 128, 4096] bf16
    fp = np.asarray(inputs["framepair_embed"][b, g0:g0 + NBLK], np.float32)
    zT = np.ascontiguousarray(
        fp.reshape(NBLK, BLK * BLK, CZ).transpose(0, 2, 1)).astype(BF)

    idx = np.asarray(inputs["rigids_to_res_idx"][b, r0:r0 + NT]).astype(np.int16)
    idx_w = np.empty((128, NT // 16), np.int16)
    for p in range(16):
        idx_w[p] = idx[p::16]
    idx_w[16:] = np.tile(idx_w[:16], (7, 1))

    btg = f("b_tgate")
    btg_const = float(btg[0]) if np.all(btg == btg[0]) else None
    has_btg = bool(np.any(btg))

    return {
        "re": np.ascontiguousarray(inputs["rigids_embed"][b, r0:r0 + NT]).astype(BF),
        "zT": zT,
        "s": np.ascontiguousarray(inputs["s"][b]).astype(np.float32),
        "idx": idx_w,
        "wq": ktile(wq_h, 4), "wk": ktile(wk_h, 4), "wv": ktile(wv_h, 4),
        "wg": ktile(wg_h, 4), "wout": ktile(inputs["Wout"], 4),
        "w1": ktile(inputs["W1"], 4), "w2": ktile(inputs["W2"], 4),
        "wb": ktile(inputs["Wb"], 8), "wada": ktile(wada_h, 3),
        "wbs": wbs_h.astype(BF),
        "bq": np.ascontiguousarray(bq_h.reshape(4, 128).T),
        "bk": np.ascontiguousarray(bk_h.reshape(4, 128).T),
        "bada": bada_h,
    }, (bool(np.any(bq_h)), bool(np.any(bk_h)), bool(np.any(f("b_ada_gate"))),
        btg_const, has_btg)


def kernel(**inputs):
    mask = np.asarray(inputs["rigids_mask"])
    if not np.all(mask == 1.0):
        print("WARNING: rigids_mask not all ones; kernel assumes ones", file=sys.stderr)

    in_maps, flags = [], None
    for core in range(NCORES):
        m, flags = prep_core_inputs(inputs, core)
        in_maps.append(m)

    nc = build(flags)
    res = run_bass_kernel_spmd(nc, in_maps, core_ids=list(range(NCORES)))

    out = np.empty((B, N, CS), np.float32)
    for core in range(NCORES):
        b = core // 4
        r0 = (core % 4) * NT
        out[b, r0:r0 + NT] = res.results[core]["out"]
    return out
